# revision 40
# baseline (speedup 1.0000x reference)
"""AdaptiveAdjacency Bass kernel for 8 TRN2 NeuronCores.

Reference computation per batch b (N=1024 nodes, H=24 hidden):
    Z   = relu(xt @ W + b)                    (N, H)
    A   = Z @ Z.T                             (N, N)  -- symmetric!
    A   = 0.5*(softmax(A, -1) + softmax(A, -2)) + I
    deg = A.sum(-1);  out = A * deg^-1/2 [row] * deg^-1/2 [col]

Math used here (exploiting symmetry of A_raw):
    E = exp(A_raw - 40)            (shift is softmax-invariant; A_raw max ~54)
    softmax(A,-2) == softmax(A,-1).T, so with r = 1/rowsum(E):
        A_sym[n,m] = E[n,m] * 0.5*(r[n]+r[m]) + I
    Fold "+I" into E:  E' = E + diag(rowsum)  =>  out = E' * C with
        C[n,m] = u[n]v[m] + v[n]u[m],  u = 0.5*r*ds,  v = ds,
        ds = degree^-1/2, degree = 1 + 0.5*colsum(r[n]*E'[n,m])
    C is rank-2 -> one K=2 matmul per output tile; single elementwise
    multiply per output element (the only full-size DVE pass).

Sharding: data-parallel over B=32 across 8 cores (4 batches each);
W/b replicated. Host pre-transposes xt to (B, F, N) fp16 so the tiny
Linear runs as a natural PE matmul (contraction over F on partitions).
"""

import numpy as np

import concourse.bass as bass
import concourse.tile as tile
from concourse import bacc, mybir
from concourse.masks import make_identity
from concourse.bass_utils import run_bass_kernel_spmd

B_FULL = 32
B_LOC = 4  # batches per core
N = 1024
F = 64
H = 24
NT = N // 128  # 8 row tiles
CK = 512  # matmul free chunk (one PSUM bank)
NCK = N // CK
KSHIFT = -40.0  # softmax shift (global constant: softmax-invariant)
N_CORES = 8

f32 = mybir.dt.float32
bf16 = mybir.dt.bfloat16
fp16 = mybir.dt.float16
AF = mybir.ActivationFunctionType
ALU = mybir.AluOpType


_TABLES_PATCHED = False


def _force_single_act_table_set():
    """All activation funcs used here (Exp, Ln, Relu, Copy/Identity) live in
    the natural_log_exp_and_others set. bacc's table-load inserter picks the
    first set containing each function, which thrashes ~2.7us per switch
    between exp_and_others and natural_log. Strip those functions from every
    other set (indices must be preserved) so one table load covers the
    whole kernel."""
    global _TABLES_PATCHED
    if _TABLES_PATCHED:
        return
    _TABLES_PATCHED = True
    import concourse.hw_specs as hw_specs

    orig = hw_specs.get_activation_tables
    keep = {
        AF.Exp,
        AF.Ln,
        AF.Relu,
        AF.Copy,
        AF.Identity,
        AF.Square,
        AF.Abs,
        AF.Sign,
        AF.MemsetZero,
        AF.Is_finite,
    }
    target = "natural_log_exp_and_others"

    def patched(module_arch):
        tables = orig(module_arch)
        if target not in tables:
            return tables
        out = {}
        for name, funcs in tables.items():
            out[name] = funcs if name == target else (funcs - keep)
        return out

    hw_specs.get_activation_tables = patched
    bacc.get_activation_tables = patched


def build_nc(
    repeat: int = 1,
    timing_trip: int | None = None,
    ablate: str | None = None,
    variant: dict | None = None,
) -> bass.Bass:
    """timing_trip=T builds a timing variant: the whole computation runs in
    an on-device For_i loop T times, writing to internal DRAM scratch with a
    tiny external output, so real device time per iteration can be measured
    by wall-clock differencing of two trip counts (fixed host/transfer costs
    cancel; code size is constant)."""
    variant = variant or {}
    _force_single_act_table_set()
    nc = bacc.Bacc()
    xtT = nc.declare_dram_parameter("xtT", [B_LOC, F, N], fp16, isOutput=False)
    Wd = nc.declare_dram_parameter("W", [F, H], fp16, isOutput=False)
    bd = nc.declare_dram_parameter("b", [H, 1], f32, isOutput=False)
    if timing_trip is None:
        outd = nc.declare_dram_parameter("out", [B_LOC, N, N], bf16, isOutput=True)
    else:
        outd = nc.dram_tensor("oscratch", [B_LOC, N, N], bf16)
        tiny_out = nc.declare_dram_parameter("out", [2, 2], f32, isOutput=True)
    # host constants (engine APs must start at partition 0, so these cannot
    # be built with sliced memsets):
    #   cst col 0/1: per-partition scale/bias for the fused colsum Ln
    #   cuv cols 0:2 = Cu, 2:4 = Cv (f32r lhsT for the log-mix matmuls)
    cstd = nc.declare_dram_parameter("cst", [2, 4], f32, isOutput=False)
    cuvd = nc.declare_dram_parameter("cuv", [2, 4], mybir.dt.float32r, isOutput=False)
    # exp bias for the fused [4,N] uvvu tail: [ln 0.5, 0, 0, ln 0.5]
    cb4d = nc.declare_dram_parameter("cb4", [4, 1], f32, isOutput=False)

    with tile.TileContext(nc) as tc:
        with (
            tc.tile_pool(name="singles", bufs=1) as singles,
            tc.tile_pool(name="zpool", bufs=B_LOC) as zpool,
            tc.tile_pool(name="epool", bufs=6 + 2 * NT) as epool,
            tc.tile_pool(name="vpool", bufs=2) as vpool,
            tc.tile_pool(name="opool", bufs=6) as opool,
            tc.tile_pool(name="tpool", bufs=2) as tpool,
            tc.tile_pool(name="apool", bufs=2, space="PSUM") as apool,
            tc.tile_pool(name="cpool", bufs=2, space="PSUM") as cpool,
            tc.tile_pool(name="cspool", bufs=1, space="PSUM") as cspool,
        ):
            # SP-ring issue order matters (~565ns sequencer cost per DMA):
            # xt0 and W first (they gate the fill chain), then the rest
            xtsbs = []
            for _b in range(B_LOC):
                xtsb = zpool.tile([F, N], fp16, tag="xt")
                xtsbs.append(xtsb)
            nc.sync.dma_start(xtsbs[0][:], xtT[0])
            wsb = singles.tile([F, H], fp16)
            nc.sync.dma_start(wsb[:], Wd[:, :])
            bsb = singles.tile([H, 1], f32)
            nc.sync.dma_start(bsb[:], bd[:, :])
            for b in range(1, B_LOC):
                nc.sync.dma_start(xtsbs[b][:], xtT[b])
            ident = singles.tile([128, 128], bf16)
            make_identity(nc, ident[:])
            cm40 = singles.tile([128, 1], f32)
            nc.gpsimd.memset(cm40[:], KSHIFT)
            # per-partition [scale, bias] for the fused colsum Ln:
            # row 0: ln(0.5*cs0 + 1.0)   row 1: ln(1.0*cs1 + 0.0)
            cstsb = singles.tile([2, 4], f32)
            nc.sync.dma_start(cstsb[:], cstd[:, :])
            cuvsb = singles.tile([2, 4], mybir.dt.float32r)
            nc.sync.dma_start(cuvsb[:], cuvd[:, :])
            cb4sb = singles.tile([4, 1], f32)
            nc.sync.dma_start(cb4sb[:], cb4d[:, :])
            # touch the ACT table set at t=0 so the 1.3us table load hides
            # in the fill instead of preceding the first exp
            scrap = singles.tile([1, 1], f32)
            nc.scalar.activation(scrap[:], cm40[0:1, 0:1], AF.Exp)

            # ---- Z^T = relu(W^T @ xt^T + b) : [H, N] fp16, all batches
            # upfront (fills otherwise-idle engines during pipeline fill and
            # keeps the batch-boundary critical path free of the Z chain) ----
            zts = []
            for b in range(B_LOC):
                xtsb = xtsbs[b]
                zpsum = apool.tile([H, N], f32, tag="ps")
                for j in range(NCK):
                    nc.tensor.matmul(
                        zpsum[:, j * CK : (j + 1) * CK],
                        wsb[:],
                        xtsb[:, j * CK : (j + 1) * CK],
                        start=True,
                        stop=True,
                    )
                zt = zpool.tile([H, N], fp16, tag="zt")
                # relu on DVE (ACT is the bottleneck engine): (Zpre + b) max 0
                nc.vector.tensor_scalar(
                    zt[:], zpsum[:], bsb[:], 0.0, ALU.add, ALU.max
                )
                # replicas at partitions 32/64/96: A_raw matmuls spread over
                # all four PE row groups (K=24 fits a 32-row group) so up to
                # four streams run concurrently in the array. Separate dst
                # tiles keep the three copies independent in Tile's
                # dependency tracking (a shared tile would serialize them
                # and stall every A_raw read behind the last copy).
                zreps = [zt]
                # batch 0 runs on groups {0,1} only: its stats phase IS the
                # pipeline fill, and waiting on the 64/96 replica DMAs would
                # push the first exp out by ~2us
                for g in (32,) if b == 0 else (32, 64, 96):
                    ztg = zpool.tile([g + H, N], fp16, tag=f"zt{g}")
                    nc.gpsimd.dma_start(ztg[g : g + H, :], zt[:])
                    zreps.append(ztg)
                zts.append(zreps)

            def stats_tile(b, i, ztpair, rowsums, r_buf, cs, e_tiles):
                """A_raw matmul, exp(+rowsum), r_i, colsum accumulate.

                E is symmetric, so only tiles 0-3 are computed full-width;
                tiles 4-7 compute just their chunk-1 half (cols 512:1024).
                The lower-left 512x512 square is mirrored from the computed
                upper-right square by XBAR-transpose DMAs into TT, and the
                missing rowsum halves of tiles 4-7 come back from the
                ones-row colsum partial (see emit_pipeline).

                The colsum runs on PLAIN E (not E' = E + diag(rowsum)):
                degree = 1.5 + 0.5*colsum(r*E) and colsum(E) = rowsum
                (by symmetry), so the diag fix stays off this critical
                chain -- it is emitted later, anywhere before the c-phase.

                The (chunk, tile-parity) pair selects one of the four PE row
                groups via Z replicas at partitions 0/32/64/96, so the two
                chunk matmuls of a tile AND adjacent tiles all overlap in
                the systolic array."""
                zreps = ztpair
                apsum = apool.tile([128, N], f32, tag="ps")
                for j in range(NCK):
                    g = (2 * (i % 2) + j) if len(zreps) == 4 else j  # 0..3
                    if len(zreps) == 2 and i == 0:
                        g = 0  # fill: tile 0 entirely on group 0, no replica
                    z = zreps[g]
                    zs = z[32 * g : 32 * g + H, :] if g else z[:, :]
                    nc.tensor.matmul(
                        apsum[:, j * CK : (j + 1) * CK],
                        zs[:, i * 128 : (i + 1) * 128],
                        zs[:, j * CK : (j + 1) * CK],
                        start=True,
                        stop=True,
                        tile_position=(32 * g, 0),
                    )
                et = epool.tile([128, N], bf16, tag="E")
                nc.scalar.activation(
                    et[:],
                    apsum[:],
                    AF.Exp,
                    bias=cm40[:],
                    accum_out=rowsums[:, i : i + 1],
                )
                if i == NT - 1:
                    # last tile: DVE's queue is ~1.3us of c-phase mults deep,
                    # but the batch tail needs r(7) -> colsum(7) -> Ln right
                    # now; exp(-ln x) on ACT runs immediately after the exp
                    lnr = vpool.tile([128, 1], f32, tag="lnr")
                    nc.scalar.activation(lnr[:], rowsums[:, i : i + 1], AF.Ln)
                    with nc.allow_low_precision("bf16 r for colsum lhsT"):
                        nc.scalar.activation(
                            r_buf[:, i, 1:2], lnr[:], AF.Exp, scale=-1.0
                        )
                else:
                    with nc.allow_low_precision("bf16 r for colsum lhsT"):
                        nc.vector.reciprocal(r_buf[:, i, 1:2], rowsums[:, i : i + 1])
                e_tiles.append(et)

            def colsum_tile(i, r_buf, cs, e_tiles):
                """Accumulate cs += [1, r_i]^T @ E_i (row 0 = plain ones sum
                = rowsum by symmetry; row 1 = r-weighted). Issued one tile
                behind the exp so the in-order PE never stalls on the
                exp/recip semaphores."""
                if ablate == "nocolsum":
                    return
                for j in range(NCK):
                    nc.tensor.matmul(
                        cs[:, j * CK : (j + 1) * CK],
                        r_buf[:, i, :],
                        e_tiles[i][:, j * CK : (j + 1) * CK],
                        start=(i == 0),
                        stop=(i == NT - 1),
                        skip_group_check=True,
                    )

            def diag_fix(i, rowsums, e_tiles):
                """E'[n,n] = E[n,n] + rowsum[n] (folds "+I" into the final
                multiply). Ordered after the colsum reads by Tile's WAR
                tracking; only needed before the c-phase. Runs on the idle
                Pool engine (SBUF-only bf16) to keep DVE free."""
                et = e_tiles[i]
                dtmp = vpool.tile([128, 128], bf16, tag="dtmp")
                nc.gpsimd.tensor_scalar_mul(dtmp[:], ident[:], rowsums[:, i : i + 1])
                nc.gpsimd.tensor_add(
                    et[:, i * 128 : (i + 1) * 128],
                    et[:, i * 128 : (i + 1) * 128],
                    dtmp[:],
                )

            def batch_tail(b, cs):
                """degree -> ds; u, v vectors (free layout).
                degree = 1 + 0.5*cs0 ; v = ds = exp(-0.5*ln(degree))
                u = 0.5*r*ds = exp(-ln(cs1) - 0.5*ln(degree))  (cs1 = 2*rowsum)
                Engine ops must be lane-aligned (partition base 0), so the
                log-domain row mixing runs on the PE (K=2 f32r matmuls
                against a tiny constant lhsT), never across partitions.
                One M=4 matmul + one [4,N] Exp makes rows (u,v,v,u); vu is
                peeled to its own base-0 tile by an ACT-ring DMA (matmul lhsT
                and rhs must share a base partition, so a slice at 2:4 can't
                pair with uv at 0:2)."""
                uv = vpool.tile([2, N], bf16, tag="uv")  # [u; v] (lhsT source)
                vu = vpool.tile([2, N], bf16, tag="vu")  # [v; u] (rhs source)
                lls = vpool.tile([2, N], mybir.dt.float32r, tag="lls")
                nc.scalar.activation(
                    lls[:], cs[:, :], AF.Ln, bias=cstsb[:, 1:2], scale=cstsb[:, 0:1]
                )
                # engine lanes are fixed (partition i -> partition i) and
                # APs below partition base 32 must start at 0, so uv and vu
                # each need their own base-0 matmul + Exp
                for coeff, dst in ((0, uv), (2, vu)):
                    lmix = apool.tile([2, N], f32, tag="ps")
                    for j in range(NCK):
                        nc.tensor.matmul(
                            lmix[:, j * CK : (j + 1) * CK],
                            cuvsb[:, coeff : coeff + 2],
                            lls[:, j * CK : (j + 1) * CK],
                            start=True,
                            stop=True,
                        )
                    # bias ln(0.5) on the u row only (cs1 is rowsum, not
                    # 2*rowsum, so u = exp(mix + ln 0.5))
                    nc.scalar.activation(
                        dst[:],
                        lmix[:],
                        AF.Exp,
                        bias=cstsb[:, 2 + coeff // 2 : 3 + coeff // 2],
                    )
                # partition-64 replica for odd c-tiles (PE row group 64):
                # [2,2N] at partitions 64-65, uv at free 0:N, vu at N:2N
                t64 = vpool.tile([66, 2 * N], bf16, tag="t64")
                nc.gpsimd.dma_start(t64[64:66, 0:N], uv[:])
                nc.gpsimd.dma_start(t64[64:66, N : 2 * N], vu[:])
                return uv, vu, t64

            def c_tile(b, i, uv, vu, t64, e_tiles, last_batch):
                """C = u v^T + v u^T (K=2 matmul), out = E' * C, DMA out."""
                osb = opool.tile([128, N], bf16, tag="o")
                if i % 2 == 0:
                    uvs, vus = uv, vu
                    voff = 0
                else:
                    uvs, vus = t64[64:66, :], t64[64:66, :]
                    voff = N
                for j in range(NCK):
                    cps = cpool.tile([128, CK], f32, tag="c")
                    nc.tensor.matmul(
                        cps[:],
                        uvs[:, i * 128 : (i + 1) * 128],
                        vus[:, voff + j * CK : voff + (j + 1) * CK],
                        start=True,
                        stop=True,
                    )
                    esrc = e_tiles[i][:, j * CK : (j + 1) * CK]
                    dmode = variant.get("drain", "pool")
                    if last_batch and j % 2 == 1 and dmode != "off":
                        # drain phase: ACT and Pool are idle, DVE is the
                        # bottleneck -- route half the final multiplies
                        # through a PSUM->SBUF copy + Pool multiply
                        csb = opool.tile([128, CK], bf16, tag="csb")
                        if dmode == "actcopy":
                            nc.scalar.activation(csb[:], cps[:], AF.Copy)
                        else:
                            nc.vector.tensor_copy(csb[:], cps[:])
                        nc.gpsimd.tensor_tensor(
                            osb[:, j * CK : (j + 1) * CK],
                            esrc,
                            csb[:],
                            ALU.mult,
                        )
                    else:
                        nc.vector.tensor_tensor(
                            osb[:, j * CK : (j + 1) * CK],
                            esrc,
                            cps[:],
                            ALU.mult,
                        )
                if ablate != "nodma":
                    nc.sync.dma_start(outd[b, i * 128 : (i + 1) * 128, :], osb[:])


            # software pipeline: batch b's stats tiles interleave with batch
            # b-1's output tiles so PE/DVE/DMA trail ACT by one phase
            def emit_pipeline(last_rep):
                prev = None
                for b in range(B_LOC):
                    rowsums = vpool.tile([128, NT], f32, tag="rowsums")
                    r_buf = vpool.tile([128, NT, 2], bf16, tag="rbuf")
                    nc.gpsimd.memset(r_buf[:], 1.0)
                    # cs[0,m] = rowsum[m] (ones row); cs[1,m] = sum r[n]E[n,m]
                    cs = cspool.tile([2, N], f32, tag="cs")
                    if ablate == "nocolsum":
                        nc.vector.memset(cs[:], 1.0)
                    e_tiles = []
                    CSLAG = 1
                    for i in range(NT):
                        stats_tile(b, i, zts[b], rowsums, r_buf, cs, e_tiles)
                        if i >= CSLAG:
                            colsum_tile(i - CSLAG, r_buf, cs, e_tiles)
                            # diag fix right behind its colsum read (Pool is
                            # idle during stats): by the tail every tile is
                            # already E', so the c-phase never waits on Pool
                            diag_fix(i - CSLAG, rowsums, e_tiles)
                        if i == NT - 1:
                            # last colsum ahead of c_tile(prev,7) in the PE
                            # queue: the Ln (batch tail) waits on it
                            colsum_tile(i, r_buf, cs, e_tiles)
                            diag_fix(i, rowsums, e_tiles)
                        if prev is not None and ablate != "statsonly":
                            c_tile(prev[0], i, *prev[1:], False)
                    uv, vu, t64 = batch_tail(b, cs)
                    prev = (b, uv, vu, t64, e_tiles)
                if ablate != "statsonly":
                    for i in range(NT):
                        c_tile(prev[0], i, *prev[1:], last_rep)

            if timing_trip is None:
                for rep in range(repeat):
                    emit_pipeline(rep == repeat - 1)
            else:
                with tc.For_i(0, timing_trip, 1):
                    emit_pipeline(False)
                tiny = singles.tile([2, 2], f32)
                nc.gpsimd.memset(tiny[:], 1.0)
                nc.sync.dma_start(tiny_out[:, :], tiny[:])

    nc.finalize()
    return nc


_NC_CACHE = None


def _get_nc() -> bass.Bass:
    global _NC_CACHE
    if _NC_CACHE is None:
        _NC_CACHE = build_nc()
    return _NC_CACHE


def _make_in_maps(xt: np.ndarray, W: np.ndarray, b: np.ndarray):
    xtT = np.ascontiguousarray(np.asarray(xt).transpose(0, 2, 1)).astype(np.float16)
    Wh = np.ascontiguousarray(np.asarray(W)).astype(np.float16)
    bh = np.ascontiguousarray(np.asarray(b)).reshape(H, 1).astype(np.float32)
    # cst cols: [Ln scale, Ln bias, uv-exp bias, vu-exp bias]
    # degree = 1.5 + 0.5*cs0 (plain-E colsum); cs1 = rowsum
    # u = exp(-0.5*ldeg - ln rs + ln 0.5), v = exp(-0.5*ldeg)
    ln_half = float(np.log(0.5))
    # cs rows: 0 = rowsum (ones colsum), 1 = r-weighted colsum w
    # Ln row 0 -> ln(rowsum); row 1 -> ln(0.5*w + 1.5) = ln(degree)
    cst = np.array(
        [[1.0, 0.0, ln_half, 0.0], [0.5, 1.5, 0.0, ln_half]], dtype=np.float32
    )
    # mix columns (u,v,v,u): u = -ln rs - 0.5 ln deg ; v = -0.5 ln deg
    cuv = np.array(
        [[-1.0, 0.0, 0.0, -1.0], [-0.5, -0.5, -0.5, -0.5]], dtype=np.float32
    )
    cb4 = np.array([[ln_half], [0.0], [0.0], [ln_half]], dtype=np.float32)
    return [
        {
            "xtT": xtT[B_LOC * k : B_LOC * (k + 1)],
            "W": Wh,
            "b": bh,
            "cst": cst,
            "cuv": cuv,
            "cb4": cb4,
        }
        for k in range(N_CORES)
    ]


def run(xt, W, b, trace: bool = False):
    """Run on 8 NeuronCores; returns (out, BassKernelResults)."""
    res = run_bass_kernel_spmd(
        _get_nc(), _make_in_maps(xt, W, b), core_ids=list(range(N_CORES)), trace=trace
    )
    out = np.concatenate(
        [np.asarray(res.results[k]["out"]) for k in range(N_CORES)], axis=0
    )
    return out.astype(np.float32, copy=False), res


def kernel(xt: np.ndarray, W: np.ndarray, b: np.ndarray) -> np.ndarray:
    out, _ = run(xt, W, b, trace=False)
    return out



# revision 41
# speedup vs baseline: 1.0831x; 1.0831x over previous
"""AdaptiveAdjacency Bass kernel for 8 TRN2 NeuronCores.

Reference computation per batch b (N=1024 nodes, H=24 hidden):
    Z   = relu(xt @ W + b)                    (N, H)
    A   = Z @ Z.T                             (N, N)  -- symmetric!
    A   = 0.5*(softmax(A, -1) + softmax(A, -2)) + I
    deg = A.sum(-1);  out = A * deg^-1/2 [row] * deg^-1/2 [col]

Math used here (exploiting symmetry of A_raw):
    E = exp(A_raw - 40)            (shift is softmax-invariant; A_raw max ~54)
    softmax(A,-2) == softmax(A,-1).T, so with r = 1/rowsum(E):
        A_sym[n,m] = E[n,m] * 0.5*(r[n]+r[m]) + I
    Fold "+I" into E:  E' = E + diag(rowsum)  =>  out = E' * C with
        C[n,m] = u[n]v[m] + v[n]u[m],  u = 0.5*r*ds,  v = ds,
        ds = degree^-1/2, degree = 1 + 0.5*colsum(r[n]*E'[n,m])
    C is rank-2 -> one K=2 matmul per output tile; single elementwise
    multiply per output element (the only full-size DVE pass).

Sharding: data-parallel over B=32 across 8 cores (4 batches each);
W/b replicated. Host pre-transposes xt to (B, F, N) fp16 so the tiny
Linear runs as a natural PE matmul (contraction over F on partitions).
"""

import numpy as np

import concourse.bass as bass
import concourse.tile as tile
from concourse import bacc, mybir
from concourse.masks import make_identity
from concourse.bass_utils import run_bass_kernel_spmd

B_FULL = 32
B_LOC = 4  # batches per core
N = 1024
F = 64
H = 24
NT = N // 128  # 8 row tiles
CK = 512  # matmul free chunk (one PSUM bank)
NCK = N // CK
KSHIFT = -40.0  # softmax shift (global constant: softmax-invariant)
N_CORES = 8

f32 = mybir.dt.float32
bf16 = mybir.dt.bfloat16
fp16 = mybir.dt.float16
AF = mybir.ActivationFunctionType
ALU = mybir.AluOpType


_TABLES_PATCHED = False


def _force_single_act_table_set():
    """All activation funcs used here (Exp, Ln, Relu, Copy/Identity) live in
    the natural_log_exp_and_others set. bacc's table-load inserter picks the
    first set containing each function, which thrashes ~2.7us per switch
    between exp_and_others and natural_log. Strip those functions from every
    other set (indices must be preserved) so one table load covers the
    whole kernel."""
    global _TABLES_PATCHED
    if _TABLES_PATCHED:
        return
    _TABLES_PATCHED = True
    import concourse.hw_specs as hw_specs

    orig = hw_specs.get_activation_tables
    keep = {
        AF.Exp,
        AF.Ln,
        AF.Relu,
        AF.Copy,
        AF.Identity,
        AF.Square,
        AF.Abs,
        AF.Sign,
        AF.MemsetZero,
        AF.Is_finite,
    }
    target = "natural_log_exp_and_others"

    def patched(module_arch):
        tables = orig(module_arch)
        if target not in tables:
            return tables
        out = {}
        for name, funcs in tables.items():
            out[name] = funcs if name == target else (funcs - keep)
        return out

    hw_specs.get_activation_tables = patched
    bacc.get_activation_tables = patched


def build_nc(
    repeat: int = 1,
    timing_trip: int | None = None,
    ablate: str | None = None,
    variant: dict | None = None,
) -> bass.Bass:
    """timing_trip=T builds a timing variant: the whole computation runs in
    an on-device For_i loop T times, writing to internal DRAM scratch with a
    tiny external output, so real device time per iteration can be measured
    by wall-clock differencing of two trip counts (fixed host/transfer costs
    cancel; code size is constant)."""
    variant = variant or {}
    _force_single_act_table_set()
    nc = bacc.Bacc()
    xtT = nc.declare_dram_parameter("xtT", [B_LOC, F, N], fp16, isOutput=False)
    Wd = nc.declare_dram_parameter("W", [F, H], fp16, isOutput=False)
    bd = nc.declare_dram_parameter("b", [H, 1], f32, isOutput=False)
    if timing_trip is None:
        outd = nc.declare_dram_parameter("out", [B_LOC, N, N], bf16, isOutput=True)
    else:
        outd = nc.dram_tensor("oscratch", [B_LOC, N, N], bf16)
        tiny_out = nc.declare_dram_parameter("out", [2, 2], f32, isOutput=True)
    # host constants (engine APs must start at partition 0, so these cannot
    # be built with sliced memsets):
    #   cst col 0/1: per-partition scale/bias for the fused colsum Ln
    #   cuv cols 0:2 = Cu, 2:4 = Cv (f32r lhsT for the log-mix matmuls)
    cstd = nc.declare_dram_parameter("cst", [2, 4], f32, isOutput=False)
    cuvd = nc.declare_dram_parameter("cuv", [2, 4], mybir.dt.float32r, isOutput=False)
    # exp bias for the fused [4,N] uvvu tail: [ln 0.5, 0, 0, ln 0.5]
    cb4d = nc.declare_dram_parameter("cb4", [4, 1], f32, isOutput=False)

    with tile.TileContext(nc) as tc:
        with (
            tc.tile_pool(name="singles", bufs=1) as singles,
            tc.tile_pool(name="zpool", bufs=B_LOC) as zpool,
            tc.tile_pool(name="epool", bufs=6 + 2 * NT) as epool,
            tc.tile_pool(name="vpool", bufs=2) as vpool,
            tc.tile_pool(name="opool", bufs=6) as opool,
            tc.tile_pool(name="tpool", bufs=2) as tpool,
            tc.tile_pool(name="apool", bufs=2, space="PSUM") as apool,
            tc.tile_pool(name="cpool", bufs=2, space="PSUM") as cpool,
            tc.tile_pool(name="cspool", bufs=1, space="PSUM") as cspool,
        ):
            # SP-ring issue order matters (~565ns sequencer cost per DMA):
            # xt0 and W first (they gate the fill chain), then the rest
            xtsbs = []
            for _b in range(B_LOC):
                xtsb = zpool.tile([F, N], fp16, tag="xt")
                xtsbs.append(xtsb)
            nc.sync.dma_start(xtsbs[0][:], xtT[0])
            wsb = singles.tile([F, H], fp16)
            nc.sync.dma_start(wsb[:], Wd[:, :])
            bsb = singles.tile([H, 1], f32)
            nc.sync.dma_start(bsb[:], bd[:, :])
            for b in range(1, B_LOC):
                nc.sync.dma_start(xtsbs[b][:], xtT[b])
            ident = singles.tile([128, 128], bf16)
            make_identity(nc, ident[:])
            cm40 = singles.tile([128, 1], f32)
            nc.gpsimd.memset(cm40[:], KSHIFT)
            # per-partition [scale, bias] for the fused colsum Ln:
            # row 0: ln(0.5*cs0 + 1.0)   row 1: ln(1.0*cs1 + 0.0)
            cstsb = singles.tile([2, 4], f32)
            nc.sync.dma_start(cstsb[:], cstd[:, :])
            cuvsb = singles.tile([2, 4], mybir.dt.float32r)
            nc.sync.dma_start(cuvsb[:], cuvd[:, :])
            cb4sb = singles.tile([4, 1], f32)
            nc.sync.dma_start(cb4sb[:], cb4d[:, :])
            # touch the ACT table set at t=0 so the 1.3us table load hides
            # in the fill instead of preceding the first exp
            scrap = singles.tile([1, 1], f32)
            nc.scalar.activation(scrap[:], cm40[0:1, 0:1], AF.Exp)

            # ---- Z^T = relu(W^T @ xt^T + b) : [H, N] fp16, all batches
            # upfront (fills otherwise-idle engines during pipeline fill and
            # keeps the batch-boundary critical path free of the Z chain) ----
            zts = []
            for b in range(B_LOC):
                xtsb = xtsbs[b]
                zpsum = apool.tile([H, N], f32, tag="ps")
                for j in range(NCK):
                    nc.tensor.matmul(
                        zpsum[:, j * CK : (j + 1) * CK],
                        wsb[:],
                        xtsb[:, j * CK : (j + 1) * CK],
                        start=True,
                        stop=True,
                    )
                zt = zpool.tile([H, N], fp16, tag="zt")
                # relu on DVE (ACT is the bottleneck engine): (Zpre + b) max 0
                nc.vector.tensor_scalar(
                    zt[:], zpsum[:], bsb[:], 0.0, ALU.add, ALU.max
                )
                # replicas at partitions 32/64/96: A_raw matmuls spread over
                # all four PE row groups (K=24 fits a 32-row group) so up to
                # four streams run concurrently in the array. Separate dst
                # tiles keep the three copies independent in Tile's
                # dependency tracking (a shared tile would serialize them
                # and stall every A_raw read behind the last copy).
                zreps = [zt]
                # batch 0 runs on groups {0,1} only: its stats phase IS the
                # pipeline fill, and waiting on the 64/96 replica DMAs would
                # push the first exp out by ~2us
                for g in (32,) if b == 0 else (32, 64, 96):
                    ztg = zpool.tile([g + H, N], fp16, tag=f"zt{g}")
                    nc.gpsimd.dma_start(ztg[g : g + H, :], zt[:])
                    zreps.append(ztg)
                zts.append(zreps)

            def stats_tile(b, i, ztpair, rowsums, r_buf, cs, e_tiles):
                """A_raw matmul, exp(+rowsum), r_i, colsum accumulate.

                E is symmetric, so only tiles 0-3 are computed full-width;
                tiles 4-7 compute just their chunk-1 half (cols 512:1024).
                The lower-left 512x512 square is mirrored from the computed
                upper-right square by XBAR-transpose DMAs into TT, and the
                missing rowsum halves of tiles 4-7 come back from the
                ones-row colsum partial (see emit_pipeline).

                The colsum runs on PLAIN E (not E' = E + diag(rowsum)):
                degree = 1.5 + 0.5*colsum(r*E) and colsum(E) = rowsum
                (by symmetry), so the diag fix stays off this critical
                chain -- it is emitted later, anywhere before the c-phase.

                The (chunk, tile-parity) pair selects one of the four PE row
                groups via Z replicas at partitions 0/32/64/96, so the two
                chunk matmuls of a tile AND adjacent tiles all overlap in
                the systolic array."""
                zreps = ztpair
                apsum = apool.tile([128, N], f32, tag="ps")
                for j in range(NCK):
                    g = (2 * (i % 2) + j) if len(zreps) == 4 else j  # 0..3
                    z = zreps[g]
                    zs = z[32 * g : 32 * g + H, :] if g else z[:, :]
                    nc.tensor.matmul(
                        apsum[:, j * CK : (j + 1) * CK],
                        zs[:, i * 128 : (i + 1) * 128],
                        zs[:, j * CK : (j + 1) * CK],
                        start=True,
                        stop=True,
                        tile_position=(32 * g, 0),
                    )
                et = epool.tile([128, N], bf16, tag="E")
                nc.scalar.activation(
                    et[:],
                    apsum[:],
                    AF.Exp,
                    bias=cm40[:],
                    accum_out=rowsums[:, i : i + 1],
                )
                if i == NT - 1:
                    # last tile: DVE's queue is ~1.3us of c-phase mults deep,
                    # but the batch tail needs r(7) -> colsum(7) -> Ln right
                    # now; exp(-ln x) on ACT runs immediately after the exp
                    lnr = vpool.tile([128, 1], f32, tag="lnr")
                    nc.scalar.activation(lnr[:], rowsums[:, i : i + 1], AF.Ln)
                    with nc.allow_low_precision("bf16 r for colsum lhsT"):
                        nc.scalar.activation(
                            r_buf[:, i, 1:2], lnr[:], AF.Exp, scale=-1.0
                        )
                else:
                    with nc.allow_low_precision("bf16 r for colsum lhsT"):
                        nc.vector.reciprocal(r_buf[:, i, 1:2], rowsums[:, i : i + 1])
                e_tiles.append(et)

            def colsum_tile(i, r_buf, cs, e_tiles):
                """Accumulate cs += [1, r_i]^T @ E_i (row 0 = plain ones sum
                = rowsum by symmetry; row 1 = r-weighted). Issued one tile
                behind the exp so the in-order PE never stalls on the
                exp/recip semaphores."""
                if ablate == "nocolsum":
                    return
                for j in range(NCK):
                    nc.tensor.matmul(
                        cs[:, j * CK : (j + 1) * CK],
                        r_buf[:, i, :],
                        e_tiles[i][:, j * CK : (j + 1) * CK],
                        start=(i == 0),
                        stop=(i == NT - 1),
                        skip_group_check=True,
                    )

            def diag_fix(i, rowsums, e_tiles):
                """E'[n,n] = E[n,n] + rowsum[n] (folds "+I" into the final
                multiply). Ordered after the colsum reads by Tile's WAR
                tracking; only needed before the c-phase. Runs on the idle
                Pool engine (SBUF-only bf16) to keep DVE free."""
                et = e_tiles[i]
                dtmp = vpool.tile([128, 128], bf16, tag="dtmp")
                nc.gpsimd.tensor_scalar_mul(dtmp[:], ident[:], rowsums[:, i : i + 1])
                nc.gpsimd.tensor_add(
                    et[:, i * 128 : (i + 1) * 128],
                    et[:, i * 128 : (i + 1) * 128],
                    dtmp[:],
                )

            def batch_tail(b, cs):
                """degree -> ds; u, v vectors (free layout).
                degree = 1 + 0.5*cs0 ; v = ds = exp(-0.5*ln(degree))
                u = 0.5*r*ds = exp(-ln(cs1) - 0.5*ln(degree))  (cs1 = 2*rowsum)
                Engine ops must be lane-aligned (partition base 0), so the
                log-domain row mixing runs on the PE (K=2 f32r matmuls
                against a tiny constant lhsT), never across partitions.
                One M=4 matmul + one [4,N] Exp makes rows (u,v,v,u); vu is
                peeled to its own base-0 tile by an ACT-ring DMA (matmul lhsT
                and rhs must share a base partition, so a slice at 2:4 can't
                pair with uv at 0:2)."""
                uv = vpool.tile([2, N], bf16, tag="uv")  # [u; v] (lhsT source)
                vu = vpool.tile([2, N], bf16, tag="vu")  # [v; u] (rhs source)
                lls = vpool.tile([2, N], mybir.dt.float32r, tag="lls")
                nc.scalar.activation(
                    lls[:], cs[:, :], AF.Ln, bias=cstsb[:, 1:2], scale=cstsb[:, 0:1]
                )
                # engine lanes are fixed (partition i -> partition i) and
                # APs below partition base 32 must start at 0, so uv and vu
                # each need their own base-0 matmul + Exp
                for coeff, dst in ((0, uv), (2, vu)):
                    lmix = apool.tile([2, N], f32, tag="ps")
                    for j in range(NCK):
                        nc.tensor.matmul(
                            lmix[:, j * CK : (j + 1) * CK],
                            cuvsb[:, coeff : coeff + 2],
                            lls[:, j * CK : (j + 1) * CK],
                            start=True,
                            stop=True,
                        )
                    # bias ln(0.5) on the u row only (cs1 is rowsum, not
                    # 2*rowsum, so u = exp(mix + ln 0.5))
                    nc.scalar.activation(
                        dst[:],
                        lmix[:],
                        AF.Exp,
                        bias=cstsb[:, 2 + coeff // 2 : 3 + coeff // 2],
                    )
                # partition-64 replica for odd c-tiles (PE row group 64):
                # [2,2N] at partitions 64-65, uv at free 0:N, vu at N:2N
                t64 = vpool.tile([66, 2 * N], bf16, tag="t64")
                nc.gpsimd.dma_start(t64[64:66, 0:N], uv[:])
                nc.gpsimd.dma_start(t64[64:66, N : 2 * N], vu[:])
                return uv, vu, t64

            def c_tile(b, i, uv, vu, t64, e_tiles, last_batch):
                """C = u v^T + v u^T (K=2 matmul), out = E' * C, DMA out."""
                osb = opool.tile([128, N], bf16, tag="o")
                if i % 2 == 0:
                    uvs, vus = uv, vu
                    voff = 0
                else:
                    uvs, vus = t64[64:66, :], t64[64:66, :]
                    voff = N
                for j in range(NCK):
                    cps = cpool.tile([128, CK], f32, tag="c")
                    nc.tensor.matmul(
                        cps[:],
                        uvs[:, i * 128 : (i + 1) * 128],
                        vus[:, voff + j * CK : voff + (j + 1) * CK],
                        start=True,
                        stop=True,
                    )
                    esrc = e_tiles[i][:, j * CK : (j + 1) * CK]
                    dmode = variant.get("drain", "actcopy")
                    if last_batch and j % 2 == 1 and dmode != "off":
                        # drain phase: ACT and Pool are idle, DVE is the
                        # bottleneck -- route half the final multiplies
                        # through a PSUM->SBUF copy + Pool multiply
                        csb = opool.tile([128, CK], bf16, tag="csb")
                        if dmode == "actcopy":
                            nc.scalar.activation(csb[:], cps[:], AF.Copy)
                        else:
                            nc.vector.tensor_copy(csb[:], cps[:])
                        nc.gpsimd.tensor_tensor(
                            osb[:, j * CK : (j + 1) * CK],
                            esrc,
                            csb[:],
                            ALU.mult,
                        )
                    else:
                        nc.vector.tensor_tensor(
                            osb[:, j * CK : (j + 1) * CK],
                            esrc,
                            cps[:],
                            ALU.mult,
                        )
                if ablate != "nodma":
                    nc.sync.dma_start(outd[b, i * 128 : (i + 1) * 128, :], osb[:])


            # software pipeline: batch b's stats tiles interleave with batch
            # b-1's output tiles so PE/DVE/DMA trail ACT by one phase
            def emit_pipeline(last_rep):
                prev = None
                for b in range(B_LOC):
                    rowsums = vpool.tile([128, NT], f32, tag="rowsums")
                    r_buf = vpool.tile([128, NT, 2], bf16, tag="rbuf")
                    nc.gpsimd.memset(r_buf[:], 1.0)
                    # cs[0,m] = rowsum[m] (ones row); cs[1,m] = sum r[n]E[n,m]
                    cs = cspool.tile([2, N], f32, tag="cs")
                    if ablate == "nocolsum":
                        nc.vector.memset(cs[:], 1.0)
                    e_tiles = []
                    CSLAG = 1
                    for i in range(NT):
                        stats_tile(b, i, zts[b], rowsums, r_buf, cs, e_tiles)
                        if i >= CSLAG:
                            colsum_tile(i - CSLAG, r_buf, cs, e_tiles)
                            # diag fix right behind its colsum read (Pool is
                            # idle during stats): by the tail every tile is
                            # already E', so the c-phase never waits on Pool
                            diag_fix(i - CSLAG, rowsums, e_tiles)
                        if i == NT - 1:
                            # last colsum ahead of c_tile(prev,7) in the PE
                            # queue: the Ln (batch tail) waits on it
                            colsum_tile(i, r_buf, cs, e_tiles)
                            diag_fix(i, rowsums, e_tiles)
                        if prev is not None and ablate != "statsonly":
                            c_tile(prev[0], i, *prev[1:], False)
                    uv, vu, t64 = batch_tail(b, cs)
                    prev = (b, uv, vu, t64, e_tiles)
                if ablate != "statsonly":
                    for i in range(NT):
                        c_tile(prev[0], i, *prev[1:], last_rep)

            if timing_trip is None:
                for rep in range(repeat):
                    emit_pipeline(rep == repeat - 1)
            else:
                with tc.For_i(0, timing_trip, 1):
                    emit_pipeline(False)
                tiny = singles.tile([2, 2], f32)
                nc.gpsimd.memset(tiny[:], 1.0)
                nc.sync.dma_start(tiny_out[:, :], tiny[:])

    nc.finalize()
    return nc


_NC_CACHE = None


def _get_nc() -> bass.Bass:
    global _NC_CACHE
    if _NC_CACHE is None:
        _NC_CACHE = build_nc()
    return _NC_CACHE


def _make_in_maps(xt: np.ndarray, W: np.ndarray, b: np.ndarray):
    xtT = np.ascontiguousarray(np.asarray(xt).transpose(0, 2, 1)).astype(np.float16)
    Wh = np.ascontiguousarray(np.asarray(W)).astype(np.float16)
    bh = np.ascontiguousarray(np.asarray(b)).reshape(H, 1).astype(np.float32)
    # cst cols: [Ln scale, Ln bias, uv-exp bias, vu-exp bias]
    # degree = 1.5 + 0.5*cs0 (plain-E colsum); cs1 = rowsum
    # u = exp(-0.5*ldeg - ln rs + ln 0.5), v = exp(-0.5*ldeg)
    ln_half = float(np.log(0.5))
    # cs rows: 0 = rowsum (ones colsum), 1 = r-weighted colsum w
    # Ln row 0 -> ln(rowsum); row 1 -> ln(0.5*w + 1.5) = ln(degree)
    cst = np.array(
        [[1.0, 0.0, ln_half, 0.0], [0.5, 1.5, 0.0, ln_half]], dtype=np.float32
    )
    # mix columns (u,v,v,u): u = -ln rs - 0.5 ln deg ; v = -0.5 ln deg
    cuv = np.array(
        [[-1.0, 0.0, 0.0, -1.0], [-0.5, -0.5, -0.5, -0.5]], dtype=np.float32
    )
    cb4 = np.array([[ln_half], [0.0], [0.0], [ln_half]], dtype=np.float32)
    return [
        {
            "xtT": xtT[B_LOC * k : B_LOC * (k + 1)],
            "W": Wh,
            "b": bh,
            "cst": cst,
            "cuv": cuv,
            "cb4": cb4,
        }
        for k in range(N_CORES)
    ]


def run(xt, W, b, trace: bool = False):
    """Run on 8 NeuronCores; returns (out, BassKernelResults)."""
    res = run_bass_kernel_spmd(
        _get_nc(), _make_in_maps(xt, W, b), core_ids=list(range(N_CORES)), trace=trace
    )
    out = np.concatenate(
        [np.asarray(res.results[k]["out"]) for k in range(N_CORES)], axis=0
    )
    return out.astype(np.float32, copy=False), res


def kernel(xt: np.ndarray, W: np.ndarray, b: np.ndarray) -> np.ndarray:
    out, _ = run(xt, W, b, trace=False)
    return out



# revision 42
# speedup vs baseline: 1.1233x; 1.0371x over previous
"""AdaptiveAdjacency Bass kernel for 8 TRN2 NeuronCores.

Reference computation per batch b (N=1024 nodes, H=24 hidden):
    Z   = relu(xt @ W + b)                    (N, H)
    A   = Z @ Z.T                             (N, N)  -- symmetric!
    A   = 0.5*(softmax(A, -1) + softmax(A, -2)) + I
    deg = A.sum(-1);  out = A * deg^-1/2 [row] * deg^-1/2 [col]

Math used here (exploiting symmetry of A_raw):
    E = exp(A_raw - 40)            (shift is softmax-invariant; A_raw max ~54)
    softmax(A,-2) == softmax(A,-1).T, so with r = 1/rowsum(E):
        A_sym[n,m] = E[n,m] * 0.5*(r[n]+r[m]) + I
    Fold "+I" into E:  E' = E + diag(rowsum)  =>  out = E' * C with
        C[n,m] = u[n]v[m] + v[n]u[m],  u = 0.5*r*ds,  v = ds,
        ds = degree^-1/2, degree = 1 + 0.5*colsum(r[n]*E'[n,m])
    C is rank-2 -> one K=2 matmul per output tile; single elementwise
    multiply per output element (the only full-size DVE pass).

Sharding: data-parallel over B=32 across 8 cores (4 batches each);
W/b replicated. Host pre-transposes xt to (B, F, N) fp16 so the tiny
Linear runs as a natural PE matmul (contraction over F on partitions).
"""

import numpy as np

import concourse.bass as bass
import concourse.tile as tile
from concourse import bacc, mybir
from concourse.masks import make_identity
from concourse.bass_utils import run_bass_kernel_spmd

B_FULL = 32
B_LOC = 4  # batches per core
N = 1024
F = 64
H = 24
NT = N // 128  # 8 row tiles
CK = 512  # matmul free chunk (one PSUM bank)
NCK = N // CK
KSHIFT = -40.0  # softmax shift (global constant: softmax-invariant)
N_CORES = 8

f32 = mybir.dt.float32
bf16 = mybir.dt.bfloat16
fp16 = mybir.dt.float16
AF = mybir.ActivationFunctionType
ALU = mybir.AluOpType


_TABLES_PATCHED = False


def _force_single_act_table_set():
    """All activation funcs used here (Exp, Ln, Relu, Copy/Identity) live in
    the natural_log_exp_and_others set. bacc's table-load inserter picks the
    first set containing each function, which thrashes ~2.7us per switch
    between exp_and_others and natural_log. Strip those functions from every
    other set (indices must be preserved) so one table load covers the
    whole kernel."""
    global _TABLES_PATCHED
    if _TABLES_PATCHED:
        return
    _TABLES_PATCHED = True
    import concourse.hw_specs as hw_specs

    orig = hw_specs.get_activation_tables
    keep = {
        AF.Exp,
        AF.Ln,
        AF.Relu,
        AF.Copy,
        AF.Identity,
        AF.Square,
        AF.Abs,
        AF.Sign,
        AF.MemsetZero,
        AF.Is_finite,
    }
    target = "natural_log_exp_and_others"

    def patched(module_arch):
        tables = orig(module_arch)
        if target not in tables:
            return tables
        out = {}
        for name, funcs in tables.items():
            out[name] = funcs if name == target else (funcs - keep)
        return out

    hw_specs.get_activation_tables = patched
    bacc.get_activation_tables = patched


def build_nc(
    repeat: int = 1,
    timing_trip: int | None = None,
    ablate: str | None = None,
    variant: dict | None = None,
) -> bass.Bass:
    """timing_trip=T builds a timing variant: the whole computation runs in
    an on-device For_i loop T times, writing to internal DRAM scratch with a
    tiny external output, so real device time per iteration can be measured
    by wall-clock differencing of two trip counts (fixed host/transfer costs
    cancel; code size is constant)."""
    variant = variant or {}
    _force_single_act_table_set()
    nc = bacc.Bacc()
    xtT = nc.declare_dram_parameter("xtT", [B_LOC, F, N], fp16, isOutput=False)
    Wd = nc.declare_dram_parameter("W", [F, H], fp16, isOutput=False)
    bd = nc.declare_dram_parameter("b", [H, 1], f32, isOutput=False)
    if timing_trip is None:
        outd = nc.declare_dram_parameter("out", [B_LOC, N, N], bf16, isOutput=True)
    else:
        outd = nc.dram_tensor("oscratch", [B_LOC, N, N], bf16)
        tiny_out = nc.declare_dram_parameter("out", [2, 2], f32, isOutput=True)
    # host constants (engine APs must start at partition 0, so these cannot
    # be built with sliced memsets):
    #   cst col 0/1: per-partition scale/bias for the fused colsum Ln
    #   cuv cols 0:2 = Cu, 2:4 = Cv (f32r lhsT for the log-mix matmuls)
    cstd = nc.declare_dram_parameter("cst", [2, 4], f32, isOutput=False)
    cuvd = nc.declare_dram_parameter("cuv", [2, 4], mybir.dt.float32r, isOutput=False)
    # exp bias for the fused [4,N] uvvu tail: [ln 0.5, 0, 0, ln 0.5]
    cb4d = nc.declare_dram_parameter("cb4", [4, 1], f32, isOutput=False)

    with tile.TileContext(nc) as tc:
        with (
            tc.tile_pool(name="singles", bufs=1) as singles,
            tc.tile_pool(name="zpool", bufs=B_LOC) as zpool,
            tc.tile_pool(name="epool", bufs=6 + 2 * NT) as epool,
            tc.tile_pool(name="vpool", bufs=3) as vpool,
            tc.tile_pool(name="opool", bufs=8) as opool,
            tc.tile_pool(name="tpool", bufs=2) as tpool,
            tc.tile_pool(name="apool", bufs=2, space="PSUM") as apool,
            tc.tile_pool(name="cpool", bufs=2, space="PSUM") as cpool,
            tc.tile_pool(name="cspool", bufs=1, space="PSUM") as cspool,
        ):
            # SP-ring issue order matters (~565ns sequencer cost per DMA):
            # xt0 and W first (they gate the fill chain), then the rest
            xtsbs = []
            for _b in range(B_LOC):
                xtsb = zpool.tile([F, N], fp16, tag="xt")
                xtsbs.append(xtsb)
            nc.sync.dma_start(xtsbs[0][:], xtT[0])
            wsb = singles.tile([F, H], fp16)
            nc.sync.dma_start(wsb[:], Wd[:, :])
            bsb = singles.tile([H, 1], f32)
            nc.sync.dma_start(bsb[:], bd[:, :])
            for b in range(1, B_LOC):
                nc.sync.dma_start(xtsbs[b][:], xtT[b])
            ident = singles.tile([128, 128], bf16)
            make_identity(nc, ident[:])
            cm40 = singles.tile([128, 1], f32)
            nc.gpsimd.memset(cm40[:], KSHIFT)
            # per-partition [scale, bias] for the fused colsum Ln:
            # row 0: ln(0.5*cs0 + 1.0)   row 1: ln(1.0*cs1 + 0.0)
            cstsb = singles.tile([2, 4], f32)
            nc.sync.dma_start(cstsb[:], cstd[:, :])
            cuvsb = singles.tile([2, 4], mybir.dt.float32r)
            nc.sync.dma_start(cuvsb[:], cuvd[:, :])
            cb4sb = singles.tile([4, 1], f32)
            nc.sync.dma_start(cb4sb[:], cb4d[:, :])
            # touch the ACT table set at t=0 so the 1.3us table load hides
            # in the fill instead of preceding the first exp
            scrap = singles.tile([1, 1], f32)
            nc.scalar.activation(scrap[:], cm40[0:1, 0:1], AF.Exp)

            # ---- Z^T = relu(W^T @ xt^T + b) : [H, N] fp16, all batches
            # upfront (fills otherwise-idle engines during pipeline fill and
            # keeps the batch-boundary critical path free of the Z chain) ----
            zts = []
            for b in range(B_LOC):
                xtsb = xtsbs[b]
                zpsum = apool.tile([H, N], f32, tag="ps")
                for j in range(NCK):
                    nc.tensor.matmul(
                        zpsum[:, j * CK : (j + 1) * CK],
                        wsb[:],
                        xtsb[:, j * CK : (j + 1) * CK],
                        start=True,
                        stop=True,
                    )
                zt = zpool.tile([H, N], fp16, tag="zt")
                # relu on DVE (ACT is the bottleneck engine): (Zpre + b) max 0
                nc.vector.tensor_scalar(
                    zt[:], zpsum[:], bsb[:], 0.0, ALU.add, ALU.max
                )
                # replicas at partitions 32/64/96: A_raw matmuls spread over
                # all four PE row groups (K=24 fits a 32-row group) so up to
                # four streams run concurrently in the array. Separate dst
                # tiles keep the three copies independent in Tile's
                # dependency tracking (a shared tile would serialize them
                # and stall every A_raw read behind the last copy).
                zreps = [zt]
                # batch 0 runs on groups {0,1} only: its stats phase IS the
                # pipeline fill, and waiting on the 64/96 replica DMAs would
                # push the first exp out by ~2us
                for g in (32,) if b == 0 else (32, 64, 96):
                    ztg = zpool.tile([g + H, N], fp16, tag=f"zt{g}")
                    nc.gpsimd.dma_start(ztg[g : g + H, :], zt[:])
                    zreps.append(ztg)
                zts.append(zreps)

            def stats_tile(b, i, ztpair, rowsums, r_buf, cs, e_tiles):
                """A_raw matmul, exp(+rowsum), r_i, colsum accumulate.

                E is symmetric, so only tiles 0-3 are computed full-width;
                tiles 4-7 compute just their chunk-1 half (cols 512:1024).
                The lower-left 512x512 square is mirrored from the computed
                upper-right square by XBAR-transpose DMAs into TT, and the
                missing rowsum halves of tiles 4-7 come back from the
                ones-row colsum partial (see emit_pipeline).

                The colsum runs on PLAIN E (not E' = E + diag(rowsum)):
                degree = 1.5 + 0.5*colsum(r*E) and colsum(E) = rowsum
                (by symmetry), so the diag fix stays off this critical
                chain -- it is emitted later, anywhere before the c-phase.

                The (chunk, tile-parity) pair selects one of the four PE row
                groups via Z replicas at partitions 0/32/64/96, so the two
                chunk matmuls of a tile AND adjacent tiles all overlap in
                the systolic array."""
                zreps = ztpair
                apsum = apool.tile([128, N], f32, tag="ps")
                for j in range(NCK):
                    g = (2 * (i % 2) + j) if len(zreps) == 4 else j  # 0..3
                    z = zreps[g]
                    zs = z[32 * g : 32 * g + H, :] if g else z[:, :]
                    nc.tensor.matmul(
                        apsum[:, j * CK : (j + 1) * CK],
                        zs[:, i * 128 : (i + 1) * 128],
                        zs[:, j * CK : (j + 1) * CK],
                        start=True,
                        stop=True,
                        tile_position=(32 * g, 0),
                    )
                et = epool.tile([128, N], bf16, tag="E")
                nc.scalar.activation(
                    et[:],
                    apsum[:],
                    AF.Exp,
                    bias=cm40[:],
                    accum_out=rowsums[:, i : i + 1],
                )
                if i == NT - 1:
                    # last tile: DVE's queue is ~1.3us of c-phase mults deep,
                    # but the batch tail needs r(7) -> colsum(7) -> Ln right
                    # now; exp(-ln x) on ACT runs immediately after the exp
                    lnr = vpool.tile([128, 1], f32, tag="lnr")
                    nc.scalar.activation(lnr[:], rowsums[:, i : i + 1], AF.Ln)
                    with nc.allow_low_precision("bf16 r for colsum lhsT"):
                        nc.scalar.activation(
                            r_buf[:, i, 1:2], lnr[:], AF.Exp, scale=-1.0
                        )
                else:
                    with nc.allow_low_precision("bf16 r for colsum lhsT"):
                        nc.vector.reciprocal(r_buf[:, i, 1:2], rowsums[:, i : i + 1])
                e_tiles.append(et)

            def colsum_tile(i, r_buf, cs, e_tiles):
                """Accumulate cs += [1, r_i]^T @ E_i (row 0 = plain ones sum
                = rowsum by symmetry; row 1 = r-weighted). Issued one tile
                behind the exp so the in-order PE never stalls on the
                exp/recip semaphores."""
                if ablate == "nocolsum":
                    return
                for j in range(NCK):
                    nc.tensor.matmul(
                        cs[:, j * CK : (j + 1) * CK],
                        r_buf[:, i, :],
                        e_tiles[i][:, j * CK : (j + 1) * CK],
                        start=(i == 0),
                        stop=(i == NT - 1),
                        skip_group_check=True,
                    )

            def diag_fix(i, rowsums, e_tiles):
                """E'[n,n] = E[n,n] + rowsum[n] (folds "+I" into the final
                multiply). Ordered after the colsum reads by Tile's WAR
                tracking; only needed before the c-phase. Runs on the idle
                Pool engine (SBUF-only bf16) to keep DVE free."""
                et = e_tiles[i]
                dtmp = vpool.tile([128, 128], bf16, tag="dtmp")
                nc.gpsimd.tensor_scalar_mul(dtmp[:], ident[:], rowsums[:, i : i + 1])
                nc.gpsimd.tensor_add(
                    et[:, i * 128 : (i + 1) * 128],
                    et[:, i * 128 : (i + 1) * 128],
                    dtmp[:],
                )

            def batch_tail(b, cs):
                """degree -> ds; u, v vectors (free layout).
                degree = 1 + 0.5*cs0 ; v = ds = exp(-0.5*ln(degree))
                u = 0.5*r*ds = exp(-ln(cs1) - 0.5*ln(degree))  (cs1 = 2*rowsum)
                Engine ops must be lane-aligned (partition base 0), so the
                log-domain row mixing runs on the PE (K=2 f32r matmuls
                against a tiny constant lhsT), never across partitions.
                One M=4 matmul + one [4,N] Exp makes rows (u,v,v,u); vu is
                peeled to its own base-0 tile by an ACT-ring DMA (matmul lhsT
                and rhs must share a base partition, so a slice at 2:4 can't
                pair with uv at 0:2)."""
                uv = vpool.tile([2, N], bf16, tag="uv")  # [u; v] (lhsT source)
                vu = vpool.tile([2, N], bf16, tag="vu")  # [v; u] (rhs source)
                lls = vpool.tile([2, N], mybir.dt.float32r, tag="lls")
                nc.scalar.activation(
                    lls[:], cs[:, :], AF.Ln, bias=cstsb[:, 1:2], scale=cstsb[:, 0:1]
                )
                # engine lanes are fixed (partition i -> partition i) and
                # APs below partition base 32 must start at 0, so uv and vu
                # each need their own base-0 matmul + Exp
                for coeff, dst in ((0, uv), (2, vu)):
                    lmix = apool.tile([2, N], f32, tag="ps")
                    for j in range(NCK):
                        nc.tensor.matmul(
                            lmix[:, j * CK : (j + 1) * CK],
                            cuvsb[:, coeff : coeff + 2],
                            lls[:, j * CK : (j + 1) * CK],
                            start=True,
                            stop=True,
                        )
                    # bias ln(0.5) on the u row only (cs1 is rowsum, not
                    # 2*rowsum, so u = exp(mix + ln 0.5))
                    nc.scalar.activation(
                        dst[:],
                        lmix[:],
                        AF.Exp,
                        bias=cstsb[:, 2 + coeff // 2 : 3 + coeff // 2],
                    )
                # partition-64 replica for odd c-tiles (PE row group 64):
                # [2,2N] at partitions 64-65, uv at free 0:N, vu at N:2N
                t64 = vpool.tile([66, 2 * N], bf16, tag="t64")
                nc.gpsimd.dma_start(t64[64:66, 0:N], uv[:])
                nc.gpsimd.dma_start(t64[64:66, N : 2 * N], vu[:])
                return uv, vu, t64

            def c_tile(b, i, uv, vu, t64, e_tiles, last_batch):
                """C = u v^T + v u^T (K=2 matmul), out = E' * C, DMA out."""
                osb = opool.tile([128, N], bf16, tag="o")
                if i % 2 == 0:
                    uvs, vus = uv, vu
                    voff = 0
                else:
                    uvs, vus = t64[64:66, :], t64[64:66, :]
                    voff = N
                for j in range(NCK):
                    cps = cpool.tile([128, CK], f32, tag="c")
                    nc.tensor.matmul(
                        cps[:],
                        uvs[:, i * 128 : (i + 1) * 128],
                        vus[:, voff + j * CK : voff + (j + 1) * CK],
                        start=True,
                        stop=True,
                    )
                    esrc = e_tiles[i][:, j * CK : (j + 1) * CK]
                    dmode = variant.get("drain", "actcopy")
                    if last_batch and j % 2 == 1 and dmode != "off":
                        # drain phase: ACT and Pool are idle, DVE is the
                        # bottleneck -- route half the final multiplies
                        # through a PSUM->SBUF copy + Pool multiply
                        csb = opool.tile([128, CK], bf16, tag="csb")
                        if dmode == "actcopy":
                            nc.scalar.activation(csb[:], cps[:], AF.Copy)
                        else:
                            nc.vector.tensor_copy(csb[:], cps[:])
                        nc.gpsimd.tensor_tensor(
                            osb[:, j * CK : (j + 1) * CK],
                            esrc,
                            csb[:],
                            ALU.mult,
                        )
                    else:
                        nc.vector.tensor_tensor(
                            osb[:, j * CK : (j + 1) * CK],
                            esrc,
                            cps[:],
                            ALU.mult,
                        )
                if ablate != "nodma":
                    nc.sync.dma_start(outd[b, i * 128 : (i + 1) * 128, :], osb[:])


            # software pipeline: batch b's stats tiles interleave with batch
            # b-1's output tiles so PE/DVE/DMA trail ACT by one phase
            def emit_pipeline(last_rep):
                prev = None
                for b in range(B_LOC):
                    rowsums = vpool.tile([128, NT], f32, tag="rowsums")
                    r_buf = vpool.tile([128, NT, 2], bf16, tag="rbuf")
                    nc.gpsimd.memset(r_buf[:], 1.0)
                    # cs[0,m] = rowsum[m] (ones row); cs[1,m] = sum r[n]E[n,m]
                    cs = cspool.tile([2, N], f32, tag="cs")
                    if ablate == "nocolsum":
                        nc.vector.memset(cs[:], 1.0)
                    e_tiles = []
                    CSLAG = 1
                    for i in range(NT):
                        stats_tile(b, i, zts[b], rowsums, r_buf, cs, e_tiles)
                        if i >= CSLAG:
                            colsum_tile(i - CSLAG, r_buf, cs, e_tiles)
                            # diag fix right behind its colsum read (Pool is
                            # idle during stats): by the tail every tile is
                            # already E', so the c-phase never waits on Pool
                            diag_fix(i - CSLAG, rowsums, e_tiles)
                        if i == NT - 1:
                            # last colsum ahead of c_tile(prev,7) in the PE
                            # queue: the Ln (batch tail) waits on it
                            colsum_tile(i, r_buf, cs, e_tiles)
                            diag_fix(i, rowsums, e_tiles)
                        if prev is not None and ablate != "statsonly":
                            c_tile(prev[0], i, *prev[1:], False)
                    uv, vu, t64 = batch_tail(b, cs)
                    prev = (b, uv, vu, t64, e_tiles)
                if ablate != "statsonly":
                    for i in range(NT):
                        c_tile(prev[0], i, *prev[1:], last_rep)

            if timing_trip is None:
                for rep in range(repeat):
                    emit_pipeline(rep == repeat - 1)
            else:
                with tc.For_i(0, timing_trip, 1):
                    emit_pipeline(False)
                tiny = singles.tile([2, 2], f32)
                nc.gpsimd.memset(tiny[:], 1.0)
                nc.sync.dma_start(tiny_out[:, :], tiny[:])

    nc.finalize()
    return nc


_NC_CACHE = None


def _get_nc() -> bass.Bass:
    global _NC_CACHE
    if _NC_CACHE is None:
        _NC_CACHE = build_nc()
    return _NC_CACHE


def _make_in_maps(xt: np.ndarray, W: np.ndarray, b: np.ndarray):
    xtT = np.ascontiguousarray(np.asarray(xt).transpose(0, 2, 1)).astype(np.float16)
    Wh = np.ascontiguousarray(np.asarray(W)).astype(np.float16)
    bh = np.ascontiguousarray(np.asarray(b)).reshape(H, 1).astype(np.float32)
    # cst cols: [Ln scale, Ln bias, uv-exp bias, vu-exp bias]
    # degree = 1.5 + 0.5*cs0 (plain-E colsum); cs1 = rowsum
    # u = exp(-0.5*ldeg - ln rs + ln 0.5), v = exp(-0.5*ldeg)
    ln_half = float(np.log(0.5))
    # cs rows: 0 = rowsum (ones colsum), 1 = r-weighted colsum w
    # Ln row 0 -> ln(rowsum); row 1 -> ln(0.5*w + 1.5) = ln(degree)
    cst = np.array(
        [[1.0, 0.0, ln_half, 0.0], [0.5, 1.5, 0.0, ln_half]], dtype=np.float32
    )
    # mix columns (u,v,v,u): u = -ln rs - 0.5 ln deg ; v = -0.5 ln deg
    cuv = np.array(
        [[-1.0, 0.0, 0.0, -1.0], [-0.5, -0.5, -0.5, -0.5]], dtype=np.float32
    )
    cb4 = np.array([[ln_half], [0.0], [0.0], [ln_half]], dtype=np.float32)
    return [
        {
            "xtT": xtT[B_LOC * k : B_LOC * (k + 1)],
            "W": Wh,
            "b": bh,
            "cst": cst,
            "cuv": cuv,
            "cb4": cb4,
        }
        for k in range(N_CORES)
    ]


def run(xt, W, b, trace: bool = False):
    """Run on 8 NeuronCores; returns (out, BassKernelResults)."""
    res = run_bass_kernel_spmd(
        _get_nc(), _make_in_maps(xt, W, b), core_ids=list(range(N_CORES)), trace=trace
    )
    out = np.concatenate(
        [np.asarray(res.results[k]["out"]) for k in range(N_CORES)], axis=0
    )
    return out.astype(np.float32, copy=False), res


def kernel(xt: np.ndarray, W: np.ndarray, b: np.ndarray) -> np.ndarray:
    out, _ = run(xt, W, b, trace=False)
    return out

